# revision 27
# baseline (speedup 1.0000x reference)
"""ButterflyConv Trainium2 kernel (fp8 DoubleRow taps, 200-col pitch).

Reference computation (per batch image):
  now = x
  for s in 0..5:
    left  = leaky(dwconv3x3(now,           W[2s])   + b[2s])
    right = leaky(dwconv3x3(now[masks[s]], W[2s+1]) + b[2s+1])
    now = left + right
  out = now + x
with leaky = LeakyReLU(0.05), SAME padding, depthwise (per-channel) 3x3 convs.

Mapping (per NeuronCore, 2 of 16 batch images):
  - 128 SBUF partitions = (batch 2) x (channel 64).
  - Free dim = image rows padded to 200 cols (4 zero cols each side; the
    DoubleRow stream step is 2 rows = 400 B, a multiple of 16), with zero
    guard rows around the image.  Single band of all 192 rows.
  - Activations and tap matrices are fp8-e4m3 on chip (the conv stack `now6`
    carries only ~12% of the output L2, so fp8's ~2% quantization is far
    inside the 2e-2 gate); the residual path is bf16.
  - 15 of 18 taps per stage run on the TensorEngine as PSUM-accumulated
    DoubleRow matmuls: each DR matmul contracts TWO 128x128 matrices against
    two fp8 streams 2 rows apart (out += w0.T@x(dy0) + w1.T@x(dy2)); the
    right branch uses 3 (dy0,dy2)-pairs + 3 zero-padded dy1 singles, the
    left branch 3 pairs with its dy1 row computed on ScalarE/VectorE and
    folded into PSUM by one identity matmul (10 PE passes/stage total).
    The channel butterfly gather is folded into the matrices.
  - leaky(v+b) is one Prelu activation pass per branch on ScalarE reading
    PSUM; left writes straight into the next-stage buffer and VectorE adds
    the right branch.  Final stage streams out = left+right+x in bf16; x is
    re-read via f32->bf16 cast DMA; host casts the output to f32.
"""

import numpy as np
import ml_dtypes

C = 64
H = 192
W_IMG = 192
NB = 6
BC = 2          # batch per core
P = 128
NCORES = 16 // BC
WPAD = 200      # 192 + 4 pad cols each side; DR stream step = 2*WPAD % 16 == 0
COL0 = 4        # first data column
NEG = 0.05

NMAT = 9                      # per stage: R 3 pairs + 3 singles, L 3 pairs
L_ACT_TAPS = [(1, 0), (1, 1)]   # (dy, dx) dy=1 row taps on ScalarE
L_DVE_TAPS = [(1, 2)]           # and one on VectorE (fp8 in, bf16 acc)
CH_ROWS = 10                  # chunk rows
CH_N = CH_ROWS * WPAD         # 1872 elems, fits 4 PSUM banks (2048)
NSTORE = CH_ROWS * W_IMG

_PROG_CACHE = {}


def _build_program():
    import concourse.bacc as bacc
    import concourse.mybir as mybir
    from concourse.tile import TileContext
    from concourse.alu_op_type import AluOpType

    f32 = mybir.dt.float32
    bf16 = mybir.dt.bfloat16
    fp8 = mybir.dt.float8e4
    prelu = mybir.ActivationFunctionType.Prelu
    DR = mybir.MatmulPerfMode.DoubleRow

    S = H + 3                     # rows: 2 guard top, image, 1 guard bottom
    SZ = (S + 2) * WPAD           # +2 slack rows: edge DR streams over-read

    nc = bacc.Bacc("TRN2", target_bir_lowering=False, debug=False,
                   enable_asserts=False, num_devices=1)

    xs_d = nc.dram_tensor("xs", [P, H * W_IMG], f32, kind="ExternalInput").ap()
    xs8_d = nc.dram_tensor("xs8", [P, H * W_IMG], fp8,
                           kind="ExternalInput").ap()
    rmat_d = nc.dram_tensor("rmat", [P, (NB * NMAT * 2 + 1) * P], fp8,
                            kind="ExternalInput").ap()
    wl_d = nc.dram_tensor("wl", [P, NB * 9], f32, kind="ExternalInput").ap()
    bl_d = nc.dram_tensor("bl", [P, NB], f32, kind="ExternalInput").ap()
    br_d = nc.dram_tensor("br", [P, NB], f32, kind="ExternalInput").ap()
    out_d = nc.dram_tensor("out", [P, H * W_IMG], bf16,
                           kind="ExternalOutput").ap()

    with TileContext(nc) as tc:
        with tc.tile_pool(name="big", bufs=1) as bigp, \
             tc.tile_pool(name="tab", bufs=1) as tabp, \
             tc.tile_pool(name="scr", bufs=2) as scrp, \
             tc.tile_pool(name="stg", bufs=2) as stgp, \
             tc.tile_pool(name="psR", bufs=1, space="PSUM") as pspR, \
             tc.tile_pool(name="psL", bufs=1, space="PSUM") as pspL:

            # --- static tables ---
            rmat_sb = tabp.tile([P, (NB * NMAT * 2 + 1) * P], fp8, tag="rmat")
            for s_ in range(NB):
                sl = slice(s_ * NMAT * 2 * P, (s_ + 1) * NMAT * 2 * P)
                nc.sync.dma_start(out=rmat_sb[:, sl], in_=rmat_d[:, sl])
            sl = slice(NB * NMAT * 2 * P, (NB * NMAT * 2 + 1) * P)
            nc.sync.dma_start(out=rmat_sb[:, sl], in_=rmat_d[:, sl])
            wl_sb = tabp.tile([P, NB * 9], f32, tag="wl")
            nc.sync.dma_start(out=wl_sb[:], in_=wl_d)
            bl_sb = tabp.tile([P, NB], f32, tag="bl")
            nc.sync.dma_start(out=bl_sb[:], in_=bl_d)
            br_sb = tabp.tile([P, NB], f32, tag="br")
            nc.sync.dma_start(out=br_sb[:], in_=br_d)

            # --- persistent fp8 image buffers (ping-pong) ---
            B = bigp.tile([P, SZ], fp8, tag="B")
            D = bigp.tile([P, SZ], fp8, tag="D")
            for t in (B, D):
                pads = (t[:, 0:S * WPAD]
                        .rearrange("p (r w) -> p r w", w=WPAD))
                nc.gpsimd.memset(pads[:, :, 0:COL0], 0.0)
                nc.gpsimd.memset(pads[:, :, COL0 + W_IMG:WPAD], 0.0)
                nc.gpsimd.memset(t[:, 0:2 * WPAD], 0.0)          # guard rows 0-1
                nc.gpsimd.memset(t[:, (S - 1) * WPAD:SZ], 0.0)   # guard+slack

            def v2d(t, row0, nrows, col0, ncols):
                return (t[:, :].rearrange("p (r w) -> p r w", w=WPAD)
                        [:, row0:row0 + nrows, col0:col0 + ncols])

            def lhsT(s, m):
                i = (s * NMAT + m) * 2 * P
                return (rmat_sb[:, i:i + 2 * P]
                        .rearrange("p (two m) -> p two m", two=2))

            id_lhsT = rmat_sb[:, NB * NMAT * 2 * P:(NB * NMAT * 2 + 1) * P]

            def dr_rhs(X, o0, sn):
                # [P, 2 (step 2*WPAD), sn] overlapping stream pair
                v = X[:, o0:o0 + 4 * WPAD].rearrange(
                    "p (two w) -> p two w", two=2)
                a = v.ap
                a[2] = [1, sn]
                v.ap = a
                return v

            # load x (fp8, host-precast), 4 slices
            n_dma = 4
            step = (H + n_dma - 1) // n_dma
            for k in range(0, H, step):
                kk = min(step, H - k)
                src = (xs8_d[:, k * W_IMG:(k + kk) * W_IMG]
                       .rearrange("p (r w) -> p r w", w=W_IMG))
                nc.sync.dma_start(out=v2d(B, 2 + k, kk, COL0, W_IMG), in_=src)

            xbuf = [B, D, B, D, B, D]
            ybuf = [D, B, D, B, D, None]
            for s in range(NB):
                X, Y = xbuf[s], ybuf[s]
                for cr0 in range(0, H, CH_ROWS):
                    cr = min(CH_ROWS, H - cr0)
                    n = cr * WPAD
                    nst = cr * W_IMG
                    e0 = (cr0 + 2) * WPAD + COL0
                    subs = []
                    so = 0
                    while so < n:
                        subs.append((so, min(512, n - so)))
                        so += 512

                    # ---- R: 6 DR matmuls (3 dy-pairs + 3 dy2 singles);
                    # L: 3 DR pairs + SBUF-acc merge.  Mats-outer so the
                    # stationary weights reload once per chunk, not per sub --
                    psR = pspR.tile([P, 2048], f32, tag="psR")
                    psL = pspL.tile([P, 2048], f32, tag="psL")
                    for m in range(6):
                        dx = m % 3
                        pair = m // 3          # 0: (dy0,dy2)  1: (zero,dy1)
                        o_base = e0 - WPAD + (dx - 1) if pair == 0 \
                            else e0 - 2 * WPAD + (dx - 1)
                        w = lhsT(s, m)
                        for so, sn in subs:
                            nc.tensor.matmul(
                                psR[:, so:so + sn], w,
                                dr_rhs(X, o_base + so, sn),
                                start=(m == 0), stop=(m == 5),
                                perf_mode=DR)

                    # L dy2-row taps off-PE: 2 on ScalarE (fp8 in, bf16 out,
                    # per-partition scale; first carries the bias), 1 on DVE
                    acc = scrp.tile([P, CH_N], bf16, tag="acc")
                    for j, (dy, dx) in enumerate(L_ACT_TAPS):
                        t = dy * 3 + dx
                        o = e0 + (dy - 1) * WPAD + (dx - 1)
                        if j == 0:
                            nc.scalar.activation(
                                out=acc[:, :n], in_=X[:, o:o + n],
                                func=mybir.ActivationFunctionType.Identity,
                                bias=bl_sb[:, s:s + 1],
                                scale=wl_sb[:, s * 9 + t:s * 9 + t + 1])
                        else:
                            ta = scrp.tile([P, CH_N], bf16, tag="ta")
                            nc.scalar.activation(
                                out=ta[:, :n], in_=X[:, o:o + n],
                                func=mybir.ActivationFunctionType.Identity,
                                bias=0.0,
                                scale=wl_sb[:, s * 9 + t:s * 9 + t + 1])
                            nc.vector.tensor_tensor(
                                out=acc[:, :n], in0=acc[:, :n],
                                in1=ta[:, :n], op=AluOpType.add)
                    for (dy, dx) in L_DVE_TAPS:
                        t = dy * 3 + dx
                        o = e0 + (dy - 1) * WPAD + (dx - 1)
                        td = scrp.tile([P, CH_N], bf16, tag="td")
                        nc.vector.tensor_scalar(
                            out=td[:, :n], in0=X[:, o:o + n],
                            scalar1=wl_sb[:, s * 9 + t:s * 9 + t + 1],
                            scalar2=0.0, op0=AluOpType.mult,
                            op1=AluOpType.add)
                        nc.vector.tensor_tensor(
                            out=acc[:, :n], in0=acc[:, :n],
                            in1=td[:, :n], op=AluOpType.add)

                    for m in range(6, 9):
                        dx = m - 6
                        o_base = e0 - WPAD + (dx - 1)   # stream0 = dy0
                        w = lhsT(s, m)
                        for so, sn in subs:
                            nc.tensor.matmul(
                                psL[:, so:so + sn], w,
                                dr_rhs(X, o_base + so, sn),
                                start=(m == 6), stop=False,
                                perf_mode=DR)
                    for so, sn in subs:
                        nc.tensor.matmul(
                            psL[:, so:so + sn], id_lhsT,
                            acc[:, so:so + sn], start=False, stop=True)

                    # ---- prelu both branches, combine, (final: +x, store) --
                    psLv = (psL[:, :n].rearrange("p (r w) -> p r w", w=WPAD)
                            [:, :, COL0:COL0 + W_IMG])
                    psRv = (psR[:, :n].rearrange("p (r w) -> p r w", w=WPAD)
                            [:, :, COL0:COL0 + W_IMG])
                    if s < NB - 1:
                        rR = scrp.tile([P, NSTORE], fp8, tag="rR")
                        rRv = (rR[:, :nst]
                               .rearrange("p (r w) -> p r w", w=W_IMG))
                        nc.scalar.activation(
                            out=rRv, in_=psRv, func=prelu,
                            bias=br_sb[:, s:s + 1], scale=1.0, alpha=NEG)
                        ysl = v2d(Y, cr0 + 2, cr, COL0, W_IMG)
                        nc.scalar.activation(
                            out=ysl, in_=psLv, func=prelu,
                            bias=0.0, scale=1.0, alpha=NEG)
                        nc.vector.tensor_tensor(
                            out=ysl, in0=ysl, in1=rRv, op=AluOpType.add)
                    else:
                        rR = scrp.tile([P, NSTORE], bf16, tag="rR5")
                        nc.scalar.activation(
                            out=(rR[:, :nst]
                                 .rearrange("p (r w) -> p r w", w=W_IMG)),
                            in_=psRv, func=prelu,
                            bias=br_sb[:, s:s + 1], scale=1.0, alpha=NEG)
                        ot = stgp.tile([P, NSTORE], bf16, tag="ot")
                        nc.scalar.activation(
                            out=(ot[:, :nst]
                                 .rearrange("p (r w) -> p r w", w=W_IMG)),
                            in_=psLv, func=prelu,
                            bias=0.0, scale=1.0, alpha=NEG)
                        nc.vector.tensor_tensor(
                            out=ot[:, :nst], in0=ot[:, :nst],
                            in1=rR[:, :nst], op=AluOpType.add)
                        xb = stgp.tile([P, NSTORE], bf16, tag="xb")
                        nc.gpsimd.dma_start(
                            out=xb[:, :nst],
                            in_=xs_d[:, cr0 * W_IMG:(cr0 + cr) * W_IMG])
                        nc.vector.tensor_tensor(
                            out=ot[:, :nst], in0=ot[:, :nst],
                            in1=xb[:, :nst], op=AluOpType.add)
                        nc.sync.dma_start(
                            out=out_d[:, cr0 * W_IMG:(cr0 + cr) * W_IMG],
                            in_=ot[:, :nst])

    nc.compile()
    return nc


def _host_tables(W, b, masks):
    """Build DR-pair weight matrices [NB*NMAT, P, 2, P] from full inputs."""
    Wt = np.asarray(W, np.float32).reshape(2 * NB, C, 3, 3)
    bt = np.asarray(b, np.float32)
    masks = np.asarray(masks, np.int64)

    # mats 0-5: right branch (3 dy-pairs + 3 zero-padded dy2 singles);
    # mats 6-8: left branch dy-pairs; dy2 row of the left branch runs on
    # ScalarE/VectorE via the wl table.
    rmat = np.zeros((NB, NMAT, P, 2, P), np.float32)
    dst_c = np.arange(C)
    for s in range(NB):
        src_r = masks[s]
        for m in range(NMAT):
            if m < 6:
                src_c, wmat, dx, pair = src_r, Wt[2 * s + 1], m % 3, m // 3
            else:
                src_c, wmat, dx, pair = dst_c, Wt[2 * s], m - 6, 0
            for two in (0, 1):
                dy = (2 * two) if pair == 0 else (1 if two == 1 else None)
                if dy is None:
                    continue
                vals = wmat[dst_c, dy, dx]
                for bb in range(BC):
                    rmat[s, m, bb * C + src_c, two, bb * C + dst_c] = vals
    np8 = ml_dtypes.float8_e4m3
    flat = rmat.reshape(NB * NMAT, P, 2 * P).transpose(1, 0, 2) \
        .reshape(P, NB * NMAT * 2 * P)
    ident = np.zeros((P, P), np.float32)
    ident[np.arange(P), np.arange(P)] = 1.0
    rmat_sb = np.ascontiguousarray(
        np.concatenate([flat, ident], axis=1)).astype(np8)

    pc = np.tile(np.arange(C), BC)
    wl = np.zeros((P, NB * 9), np.float32)
    bl = np.zeros((P, NB), np.float32)
    br = np.zeros((P, NB), np.float32)
    for s in range(NB):
        for t in range(9):
            wl[:, s * 9 + t] = Wt[2 * s, pc, t // 3, t % 3]
        bl[:, s] = bt[2 * s, pc]
        br[:, s] = bt[2 * s + 1, pc]
    return rmat_sb, wl, bl, br


def _get_prog():
    key = (H, CH_ROWS, "v5")
    if key not in _PROG_CACHE:
        _PROG_CACHE[key] = _build_program()
    return _PROG_CACHE[key]


def _run_on_hw(nc, in_maps, trace=False, **kw):
    from concourse import bass_utils
    return bass_utils.run_bass_kernel_spmd(
        nc, in_maps, core_ids=list(range(len(in_maps))), trace=trace, **kw)


def _make_in_maps(x, W, b, masks):
    rmat_sb, wl, bl, br = _host_tables(W, b, masks)
    x = np.asarray(x, np.float32)
    np8 = ml_dtypes.float8_e4m3
    nb_total = x.shape[0]
    in_maps = []
    for k in range(0, nb_total, BC):
        xs = np.ascontiguousarray(x[k:k + BC].reshape(BC * C, H * W_IMG))
        in_maps.append({"xs": xs, "xs8": xs.astype(np8), "rmat": rmat_sb,
                        "wl": wl, "bl": bl, "br": br})
    return in_maps


def kernel(x, W, b, masks):
    nc = _get_prog()
    in_maps = _make_in_maps(x, W, b, masks)
    res = _run_on_hw(nc, in_maps)
    outs = [np.asarray(r["out"]).astype(np.float32)
            .reshape(BC, C, H, W_IMG) for r in res.results]
    return np.concatenate(outs, axis=0)
